# revision 1
# baseline (speedup 1.0000x reference)
"""GNN message-passing (MGN mailbox sum + Linear + indeg blend) on 8 Trainium2 cores.

Reference semantics (for full inputs h[40000,128], W[128,128], b[128],
src/dst[640000]):
    agg     = segment_sum(h[src], dst, 40000)
    updated = agg @ W.T + b
    out     = where(indeg > 0, updated, h)

Sharding (per the problem's sharding hint): edges and their *gathered
features* are sharded across the 8 cores by destination-node range; the
Linear weight is replicated. Each core owns 5120 destination nodes (40
windows of 128). The host buckets edges by destination window (a sort by
dst) and ships each core the pre-gathered edge features h[src] (bf16) in a
fixed [window, tile, slot] layout, plus per-slot one-hot column indices.

Device compute per window w (40 per core):
    O_w   = onehot(dst_local)          # GpSimd local_scatter (2 halves)
    aggT  = sum_t stage_t.T @ O_t      # PE, PSUM accumulate   [128f, 128n]
    updT  = W @ aggT                   # PE (replicated W)     [128o, 128n]
    updT += b                          # ACT Identity+bias
    outT  = where(maskT, updT, hT)     # DVE copy_predicated, in-place in the
                                       #   resident hT buffer
Everything stays feature-major (no on-chip transposes); the host
transposes each core's [128, 5120] result back at the end.

Slots beyond a window's edge count get one-hot column -1 (not written ->
zero one-hot row). If a window exceeds the T*128 slot capacity (6-sigma
event), the affected destination nodes are recomputed exactly on the host
and patched into the output.
"""

import sys

sys.path.insert(0, "/opt/trn_rl_repo")

import numpy as np
import ml_dtypes

import concourse.bacc as bacc
import concourse.mybir as mybir
import concourse.tile as tile
from concourse.bass_utils import run_bass_kernel_spmd

BF16 = ml_dtypes.bfloat16

# problem geometry (hardcoded per spec)
N_NODES = 40000
N_EDGES = 640000
HID = 128
P = 128

N_CORES = 8
PAD_NODES = 40960           # 8 cores x 40 windows x 128 nodes
NPC = PAD_NODES // N_CORES  # 5120 nodes per core
WPC = NPC // P              # 40 windows per core
T = 17                      # edge tiles per window (capacity T*128 = 2176, mean 2048)
THA = 9                     # tiles in one-hot half A
THB = T - THA               # tiles in one-hot half B
NIXA = THA + 1              # local_scatter num_idxs, half A (even)
NIXB = THB                  # half B is already even
NIX2 = NIXA + NIXB          # per-window col-index entries
GRP = 2                     # windows fused per Linear/bias/blend batch (512 cols)

_NC_CACHE = {}


def _build_nc():
    """Build the (shared, SPMD) bass program. Same program runs on all 8 cores."""
    key = "v7"
    if key in _NC_CACHE:
        return _NC_CACHE[key]
    f32 = mybir.dt.float32
    bf16 = mybir.dt.bfloat16
    i16 = mybir.dt.int16
    nc = bacc.Bacc(None, target_bir_lowering=False)

    stage = nc.declare_dram_parameter("stage", [P, WPC * T * P], bf16, isOutput=False)
    colix = nc.declare_dram_parameter("colix", [P, WPC * NIX2], i16, isOutput=False)
    dl = nc.declare_dram_parameter("dl", [P, WPC * T], bf16, isOutput=False)
    iota = nc.declare_dram_parameter("iota", [P, P], bf16, isOutput=False)
    wt = nc.declare_dram_parameter("wt", [P, P], bf16, isOutput=False)
    b2 = nc.declare_dram_parameter("b2", [P, 1], f32, isOutput=False)
    hT = nc.declare_dram_parameter("hT", [P, NPC], f32, isOutput=False)
    maskT = nc.declare_dram_parameter("maskT", [P, NPC], mybir.dt.uint8, isOutput=False)
    outT = nc.declare_dram_parameter("outT", [P, NPC], f32, isOutput=True)

    with tile.TileContext(nc) as tc:
        with (
            tc.tile_pool(name="const", bufs=1) as constp,
            tc.tile_pool(name="big", bufs=1) as bigp,
            tc.tile_pool(name="stagep", bufs=5) as stagep,
            tc.tile_pool(name="onehotp", bufs=8) as onehotp,
            tc.tile_pool(name="smallp", bufs=6) as smallp,
            tc.tile_pool(name="psA", bufs=4, space="PSUM") as psA,
            tc.tile_pool(name="psB", bufs=2, space="PSUM") as psB,
        ):
            wt_t = constp.tile([P, P], bf16)
            nc.sync.dma_start(out=wt_t[:], in_=wt[:])
            b2_t = constp.tile([P, 1], f32)
            nc.sync.dma_start(out=b2_t[:], in_=b2[:])
            ones_t = constp.tile([P, NIXA], bf16)
            nc.vector.memset(ones_t[:], 1.0)
            cix_t = constp.tile([P, WPC * NIX2], i16)
            nc.sync.dma_start(out=cix_t[:], in_=colix[:])
            iota_t = constp.tile([P, P], bf16)
            nc.sync.dma_start(out=iota_t[:], in_=iota[:])
            dl_t = constp.tile([P, WPC * T], bf16)
            nc.sync.dma_start(out=dl_t[:], in_=dl[:])

            hT_buf = bigp.tile([P, NPC], f32)
            nc.sync.dma_start(out=hT_buf[:], in_=hT[:])
            mk_buf = bigp.tile([P, NPC], mybir.dt.uint8)
            nc.sync.dma_start(out=mk_buf[:], in_=maskT[:])

            for w in range(WPC):
                st = stagep.tile([P, T * P], bf16, tag="stage")
                nc.sync.dma_start(out=st[:], in_=stage[:, w * T * P : (w + 1) * T * P])

                oh_ap = []
                for half, (thn, base_ix, nix) in enumerate(
                    [(THA, 0, NIXA), (THB, NIXA, NIXB)]
                ):
                    if ((w * 2 + half) * 17) % 40 < 17:
                        o = onehotp.tile([P, thn * P], bf16, tag=f"ohd{half}")
                        tb = w * T + (0 if half == 0 else THA)
                        nc.vector.tensor_tensor(
                            out=o[:].rearrange("p (t f) -> p t f", f=P),
                            in0=dl_t[:, tb : tb + thn, None].to_broadcast(
                                [P, thn, P]
                            ),
                            in1=iota_t[:, None, :].to_broadcast([P, thn, P]),
                            op=mybir.AluOpType.is_equal,
                        )
                    else:
                        o = onehotp.tile([P, thn * P], bf16, tag=f"ohg{half}")
                        nc.gpsimd.local_scatter(
                            out_ap=o[:],
                            data_ap=ones_t[:, :nix],
                            idxs_ap=cix_t[
                                :, w * NIX2 + base_ix : w * NIX2 + base_ix + nix
                            ],
                            channels=P,
                            num_elems=thn * P,
                            num_idxs=nix,
                        )
                    oh_ap.append((o, 0))

                paggT = psA.tile([P, P], f32, tag="paggT")
                for t in range(T):
                    o, base = oh_ap[0] if t < THA else oh_ap[1]
                    tl = t if t < THA else t - THA
                    nc.tensor.matmul(
                        out=paggT[:],
                        lhsT=st[:, t * P : (t + 1) * P],
                        rhs=o[:, base + tl * P : base + (tl + 1) * P],
                        start=(t == 0),
                        stop=(t == T - 1),
                    )
                wi = w % GRP
                if wi == 0:
                    aggT4 = smallp.tile([P, GRP * P], bf16, tag="aggT")
                nc.scalar.copy(
                    out=aggT4[:, wi * P : (wi + 1) * P], in_=paggT[:]
                )

                if wi == GRP - 1:
                    g0 = (w - GRP + 1) * P
                    pupdT = psB.tile([P, GRP * P], f32, tag="pupdT")
                    nc.tensor.matmul(
                        out=pupdT[:], lhsT=wt_t[:], rhs=aggT4[:], start=True, stop=True
                    )
                    updT_s = smallp.tile([P, GRP * P], f32, tag="updT")
                    nc.scalar.activation(
                        out=updT_s[:],
                        in_=pupdT[:],
                        func=mybir.ActivationFunctionType.Identity,
                        bias=b2_t[:, :1],
                    )
                    nc.vector.copy_predicated(
                        hT_buf[:, g0 : g0 + GRP * P],
                        mk_buf[:, g0 : g0 + GRP * P],
                        updT_s[:],
                    )

            nc.sync.dma_start(out=outT[:], in_=hT_buf[:])

    nc.finalize()
    _NC_CACHE[key] = nc
    return nc


def kernel(h, W, b, src, dst):
    h = np.ascontiguousarray(np.asarray(h, dtype=np.float32))
    W = np.ascontiguousarray(np.asarray(W, dtype=np.float32))
    b = np.ascontiguousarray(np.asarray(b, dtype=np.float32))
    src = np.asarray(src).astype(np.int64)
    dst = np.asarray(dst).astype(np.int64)
    n, hid = h.shape
    assert (n, hid) == (N_NODES, HID)

    h_pad = np.zeros((PAD_NODES + 1, HID), np.float32)  # +1: row PAD_NODES = zero row
    h_pad[:N_NODES] = h
    h_pad_bf = h_pad.astype(BF16)

    # ---- host-side sharding: bucket edges by dst window, fixed-capacity slots
    order = np.argsort(dst, kind="stable")
    dst_s = dst[order]
    src_s = src[order]
    win_bounds = np.searchsorted(dst_s, np.arange(0, PAD_NODES + P, P))
    cap = T * P

    n_win = PAD_NODES // P  # 320
    spill_nodes = []
    slot_src = np.full((n_win, cap), PAD_NODES, np.int64)  # default: zero row
    slot_dl = np.full((n_win, cap), -1, np.int64)          # -1: empty slot
    for wgl in range(n_win):
        lo, hi = win_bounds[wgl], win_bounds[wgl + 1]
        cnt = hi - lo
        take = min(cnt, cap)
        slot_src[wgl, :take] = src_s[lo : lo + take]
        slot_dl[wgl, :take] = dst_s[lo : lo + take] - wgl * P
        if cnt > cap:
            spill_nodes.append(np.unique(dst_s[lo + cap : hi]))

    indeg = np.bincount(dst, minlength=PAD_NODES)

    # one-hot column indices per slot: col = (tile % TH) * 128 + dst_local
    # shipped layout: [P, WPC * 2 * NIX] int16; per (window, half): NIX entries
    # per partition (tile-within-half 0..TH-1, then one padding -1)
    sl = slot_dl.reshape(n_win, T, P)  # [win, tile, part]
    colix_all = np.full((n_win, NIX2, P), -1, np.int64)
    tlA = sl[:, :THA, :]
    colix_all[:, :THA, :] = np.where(
        tlA >= 0, (np.arange(THA)[None, :, None]) * P + tlA, -1
    )
    tlB = sl[:, THA:, :]
    colix_all[:, NIXA : NIXA + THB, :] = np.where(
        tlB >= 0, (np.arange(THB)[None, :, None]) * P + tlB, -1
    )

    WT = np.ascontiguousarray(W.T).astype(BF16)
    b2 = np.ascontiguousarray(b[:, None])
    iota_np = np.tile(np.arange(P, dtype=np.float32), (P, 1)).astype(BF16)

    in_maps = []
    for c in range(N_CORES):
        wsl = slice(c * WPC, (c + 1) * WPC)
        rows = h_pad_bf[slot_src[wsl]]  # [WPC, cap, HID]
        rows = rows.reshape(WPC, T, P, HID)
        stage_np = np.ascontiguousarray(
            rows.transpose(2, 0, 1, 3).reshape(P, WPC * T * P)
        )
        colix_np = np.ascontiguousarray(
            colix_all[wsl].transpose(2, 0, 1).reshape(P, WPC * NIX2)
        ).astype(np.int16)
        dl_np = np.ascontiguousarray(
            np.where(slot_dl[wsl] >= 0, slot_dl[wsl], 255)
            .reshape(WPC, T, P)
            .transpose(2, 0, 1)
            .reshape(P, WPC * T)
            .astype(np.float32)
        ).astype(BF16)
        hT_np = np.ascontiguousarray(h_pad[c * NPC : (c + 1) * NPC, :].T)
        maskT_np = np.ascontiguousarray(
            np.broadcast_to(
                (indeg[c * NPC : (c + 1) * NPC] > 0).astype(np.uint8)[None, :],
                (P, NPC),
            )
        )
        in_maps.append(
            {
                "stage": stage_np,
                "colix": colix_np,
                "dl": dl_np,
                "iota": iota_np,
                "wt": WT,
                "b2": b2,
                "hT": hT_np,
                "maskT": maskT_np,
            }
        )

    nc = _build_nc()
    res = run_bass_kernel_spmd(nc, in_maps, core_ids=list(range(N_CORES)))

    out = np.concatenate([res.results[c]["outT"].T for c in range(N_CORES)], axis=0)
    out = np.ascontiguousarray(out[:N_NODES])

    # ---- host patch for (statistically negligible) window-capacity spill
    if spill_nodes:
        nodes = np.unique(np.concatenate(spill_nodes))
        nodes = nodes[nodes < N_NODES]
        if nodes.size:
            sel = np.isin(dst, nodes)
            agg = np.zeros((nodes.size, HID), np.float32)
            remap = {int(v): i for i, v in enumerate(nodes)}
            np.add.at(agg, [remap[int(d)] for d in dst[sel]], h[src[sel]])
            out[nodes] = agg @ W.T + b

    return out



# revision 5
# speedup vs baseline: 1.0607x; 1.0607x over previous
"""GNN message-passing (MGN mailbox sum + Linear + indeg blend) on 8 Trainium2 cores.

Reference semantics (full inputs h[40000,128], W[128,128], b[128],
src/dst[640000]):
    agg     = segment_sum(h[src], dst, 40000)
    updated = agg @ W.T + b
    out     = where(indeg > 0, updated, h)

Sharding (per the problem's sharding hint): edges and their *gathered
features* are sharded across the 8 cores by destination-node range; the
Linear weight is replicated. Each core owns 5120 destination nodes (40
windows of 128). The host buckets edges by destination window (a sort by
dst) and ships each core the pre-gathered edge features h[src] in a fixed
[window, tile, slot] layout, quantized to fp8e4 with per-destination
error-diffusion (the residual carry telescopes within each dst's edge
run, so the segment-sum error is ~one quantization step instead of
sqrt(indeg) steps), plus per-slot dst-local ids / one-hot column indices.

Device compute per window w (40 per core):
    O_w   = onehot(dst_local)          # DVE tensor_scalar is_equal (4x mode)
                                       #   or GpSimd local_scatter (2 halves)
    aggT  = sum_t stage_t.T @ O_t      # PE, fp8 lhsT x bf16 rhs, PSUM f32
    updT  = W @ aggT                   # PE (replicated W, bf16)
    updT += b                          # ACT Identity+bias -> bf16
    outT[:, w] = updT                  # DMA out per window group
Nodes with indeg == 0 (expected ~0 of 40000 at mean indeg 16) keep h;
the host patches them after the gather, along with any window-capacity
spill (6-sigma event) recomputed exactly on the host.
"""

import sys

sys.path.insert(0, "/opt/trn_rl_repo")

import numpy as np
import ml_dtypes

import concourse.bacc as bacc
import concourse.mybir as mybir
import concourse.tile as tile
from concourse.bass_utils import run_bass_kernel_spmd

BF16 = ml_dtypes.bfloat16
FP8 = ml_dtypes.float8_e4m3

# problem geometry (hardcoded per spec)
N_NODES = 40000
N_EDGES = 640000
HID = 128
P = 128

N_CORES = 8
PAD_NODES = 40960           # 8 cores x 40 windows x 128 nodes
NPC = PAD_NODES // N_CORES  # 5120 nodes per core
WPC = NPC // P              # 40 windows per core
T = 17                      # edge tiles per window (capacity T*128 = 2176, mean 2048)
THA = 9                     # tiles in one-hot half A (gpsimd path)
THB = T - THA
NIXA = THA + 1              # local_scatter num_idxs, half A (even)
NIXB = THB
NIX2 = NIXA + NIXB
GRP = 2                     # windows fused per Linear/bias batch (256 cols)

_NC_CACHE = {}


def _dve_window(w: int) -> bool:
    """One-hot engine split: ~24 windows on DVE, ~16 on GpSimd."""
    return w % 5 != 0 and w % 5 != 3


def _build_nc():
    key = "v8"
    if key in _NC_CACHE:
        return _NC_CACHE[key]
    f32 = mybir.dt.float32
    bf16 = mybir.dt.bfloat16
    fp8 = mybir.dt.float8e4
    i16 = mybir.dt.int16
    nc = bacc.Bacc(None, target_bir_lowering=False)

    stage = nc.declare_dram_parameter("stage", [P, WPC * T * P], fp8, isOutput=False)
    colix = nc.declare_dram_parameter("colix", [P, WPC * NIX2], i16, isOutput=False)
    dl = nc.declare_dram_parameter("dl", [P, WPC * T], f32, isOutput=False)
    iota = nc.declare_dram_parameter("iota", [P, P], bf16, isOutput=False)
    wt = nc.declare_dram_parameter("wt", [P, P], bf16, isOutput=False)
    b2 = nc.declare_dram_parameter("b2", [P, 1], f32, isOutput=False)
    outT = nc.declare_dram_parameter("outT", [P, NPC], bf16, isOutput=True)

    with tile.TileContext(nc) as tc:
        with (
            tc.tile_pool(name="const", bufs=1) as constp,
            tc.tile_pool(name="stagep", bufs=6) as stagep,
            tc.tile_pool(name="onehotp", bufs=4) as onehotp,
            tc.tile_pool(name="smallp", bufs=6) as smallp,
            tc.tile_pool(name="psA", bufs=4, space="PSUM") as psA,
            tc.tile_pool(name="psB", bufs=2, space="PSUM") as psB,
        ):
            wt_t = constp.tile([P, P], bf16)
            nc.sync.dma_start(out=wt_t[:], in_=wt[:])
            b2_t = constp.tile([P, 1], f32)
            nc.sync.dma_start(out=b2_t[:], in_=b2[:])
            ones_t = constp.tile([P, NIXA], bf16)
            nc.vector.memset(ones_t[:], 1.0)
            cix_t = constp.tile([P, WPC * NIX2], i16)
            nc.sync.dma_start(out=cix_t[:], in_=colix[:])
            iota_t = constp.tile([P, P], bf16)
            nc.sync.dma_start(out=iota_t[:], in_=iota[:])
            dl_t = constp.tile([P, WPC * T], f32)
            nc.sync.dma_start(out=dl_t[:], in_=dl[:])

            for w in range(WPC):
                st = stagep.tile([P, T * P], fp8, tag="stage")
                nc.sync.dma_start(out=st[:], in_=stage[:, w * T * P : (w + 1) * T * P])

                oh = onehotp.tile([P, T * P], bf16, tag="oh")
                if _dve_window(w):
                    for t in range(T):
                        nc.vector.tensor_scalar(
                            out=oh[:, t * P : (t + 1) * P],
                            in0=iota_t[:],
                            scalar1=dl_t[:, w * T + t : w * T + t + 1],
                            scalar2=None,
                            op0=mybir.AluOpType.is_equal,
                        )
                else:
                    for thn, base_ix, nix, off in (
                        (THA, 0, NIXA, 0),
                        (THB, NIXA, NIXB, THA * P),
                    ):
                        nc.gpsimd.local_scatter(
                            out_ap=oh[:, off : off + thn * P],
                            data_ap=ones_t[:, :nix],
                            idxs_ap=cix_t[
                                :, w * NIX2 + base_ix : w * NIX2 + base_ix + nix
                            ],
                            channels=P,
                            num_elems=thn * P,
                            num_idxs=nix,
                        )

                paggT = psA.tile([P, P], f32, tag="paggT")
                for t in range(T):
                    nc.tensor.matmul(
                        out=paggT[:],
                        lhsT=st[:, t * P : (t + 1) * P],
                        rhs=oh[:, t * P : (t + 1) * P],
                        start=(t == 0),
                        stop=(t == T - 1),
                    )
                wi = w % GRP
                if wi == 0:
                    aggT4 = smallp.tile([P, GRP * P], bf16, tag="aggT")
                nc.scalar.copy(out=aggT4[:, wi * P : (wi + 1) * P], in_=paggT[:])

                if wi == GRP - 1:
                    g0 = (w - GRP + 1) * P
                    pupdT = psB.tile([P, GRP * P], f32, tag="pupdT")
                    nc.tensor.matmul(
                        out=pupdT[:], lhsT=wt_t[:], rhs=aggT4[:], start=True, stop=True
                    )
                    updT_s = smallp.tile([P, GRP * P], bf16, tag="updT")
                    nc.scalar.activation(
                        out=updT_s[:],
                        in_=pupdT[:],
                        func=mybir.ActivationFunctionType.Identity,
                        bias=b2_t[:, :1],
                    )
                    nc.sync.dma_start(
                        out=outT[:, g0 : g0 + GRP * P], in_=updT_s[:]
                    )

    nc.finalize()
    _NC_CACHE[key] = nc
    return nc


def kernel(h, W, b, src, dst):
    h = np.ascontiguousarray(np.asarray(h, dtype=np.float32))
    W = np.ascontiguousarray(np.asarray(W, dtype=np.float32))
    b = np.ascontiguousarray(np.asarray(b, dtype=np.float32))
    src = np.asarray(src).astype(np.int64)
    dst = np.asarray(dst).astype(np.int64)
    n, hid = h.shape
    assert (n, hid) == (N_NODES, HID)

    # ---- host-side sharding: bucket edges by dst window, fixed-capacity slots
    order = np.argsort(dst, kind="stable")
    dst_s = dst[order]
    src_s = src[order]
    win_bounds = np.searchsorted(dst_s, np.arange(0, PAD_NODES + P, P))
    cap = T * P
    n_win = PAD_NODES // P  # 320

    # fp8 quantization of gathered rows with per-destination error diffusion:
    # within each dst's contiguous run of edges, quantize v + carry and push
    # the residual onto the next edge; the run's sum error telescopes to the
    # final carry (~one fp8 step) instead of accumulating across edges.
    vals = h[src_s]  # [E, HID] f32, dst-sorted
    starts = np.searchsorted(dst_s, np.arange(N_NODES))
    counts = np.bincount(dst_s, minlength=N_NODES)
    q = np.empty((N_EDGES, HID), FP8)
    carry = np.zeros((N_NODES, HID), np.float32)
    for k in range(int(counts.max())):
        sel = counts > k
        pos = starts[sel] + k
        v = vals[pos] + carry[sel]
        qk = v.astype(FP8)
        q[pos] = qk
        carry[sel] = v - qk.astype(np.float32)

    spill_nodes = []
    slot_q = np.zeros((n_win, cap, HID), FP8)
    slot_dl = np.full((n_win, cap), -1, np.int64)
    for wgl in range(n_win):
        lo, hi = win_bounds[wgl], win_bounds[wgl + 1]
        cnt = hi - lo
        take = min(cnt, cap)
        slot_q[wgl, :take] = q[lo : lo + take]
        slot_dl[wgl, :take] = dst_s[lo : lo + take] - wgl * P
        if cnt > cap:
            spill_nodes.append(np.unique(dst_s[lo + cap : hi]))

    indeg = np.bincount(dst, minlength=PAD_NODES)

    # one-hot column indices per slot (gpsimd local_scatter path):
    # col = (tile % TH) * 128 + dst_local, per (window, half)
    sl = slot_dl.reshape(n_win, T, P)
    colix_all = np.full((n_win, NIX2, P), -1, np.int64)
    tlA = sl[:, :THA, :]
    colix_all[:, :THA, :] = np.where(
        tlA >= 0, (np.arange(THA)[None, :, None]) * P + tlA, -1
    )
    tlB = sl[:, THA:, :]
    colix_all[:, NIXA : NIXA + THB, :] = np.where(
        tlB >= 0, (np.arange(THB)[None, :, None]) * P + tlB, -1
    )

    WT = np.ascontiguousarray(W.T).astype(BF16)
    b2 = np.ascontiguousarray(b[:, None])
    iota_np = np.tile(np.arange(P, dtype=np.float32), (P, 1)).astype(BF16)

    in_maps = []
    for c in range(N_CORES):
        wsl = slice(c * WPC, (c + 1) * WPC)
        rows = slot_q[wsl].reshape(WPC, T, P, HID)
        stage_np = np.ascontiguousarray(
            rows.transpose(2, 0, 1, 3).reshape(P, WPC * T * P)
        )
        colix_np = np.ascontiguousarray(
            colix_all[wsl].transpose(2, 0, 1).reshape(P, WPC * NIX2)
        ).astype(np.int16)
        dl_np = np.ascontiguousarray(
            np.where(slot_dl[wsl] >= 0, slot_dl[wsl], 255)
            .reshape(WPC, T, P)
            .transpose(2, 0, 1)
            .reshape(P, WPC * T)
            .astype(np.float32)
        )
        in_maps.append(
            {
                "stage": stage_np,
                "colix": colix_np,
                "dl": dl_np,
                "iota": iota_np,
                "wt": WT,
                "b2": b2,
            }
        )

    nc = _build_nc()
    res = run_bass_kernel_spmd(nc, in_maps, core_ids=list(range(N_CORES)))

    out = np.concatenate(
        [res.results[c]["outT"].T.astype(np.float32) for c in range(N_CORES)], axis=0
    )
    out = np.ascontiguousarray(out[:N_NODES])

    # nodes with no incoming edge keep their input feature
    zi = np.flatnonzero(indeg[:N_NODES] == 0)
    if zi.size:
        out[zi] = h[zi]

    # ---- host patch for (statistically negligible) window-capacity spill
    if spill_nodes:
        nodes = np.unique(np.concatenate(spill_nodes))
        nodes = nodes[nodes < N_NODES]
        if nodes.size:
            sel = np.isin(dst, nodes)
            agg = np.zeros((nodes.size, HID), np.float32)
            remap = {int(v): i for i, v in enumerate(nodes)}
            np.add.at(agg, [remap[int(d)] for d in dst[sel]], h[src[sel]])
            out[nodes] = agg @ W.T + b

    return out


# revision 11
# speedup vs baseline: 1.4011x; 1.3210x over previous
"""GNN message-passing (MGN mailbox sum + Linear + indeg blend) on 8 Trainium2 cores.

Reference semantics (full inputs h[40000,128], W[128,128], b[128],
src/dst[640000]):
    agg     = segment_sum(h[src], dst, 40000)
    updated = agg @ W.T + b
    out     = where(indeg > 0, updated, h)

Sharding (per the problem's sharding hint): edges and their *gathered
features* are sharded across the 8 cores by destination-node range; the
Linear weight is replicated. Each core owns 5120 destination nodes (40
windows of 128). The host buckets edges by destination window (a sort by
dst) and ships each core the pre-gathered edge features h[src] in a fixed
[window, tile, slot] layout, quantized to fp8e4 with per-destination
error-diffusion (the residual carry telescopes within each dst's edge
run, so the segment-sum error is ~one quantization step instead of
sqrt(indeg) steps), plus per-slot dst-local ids / one-hot column indices.

Device compute per window w (40 per core):
    O_w   = onehot(dst_local)          # DVE tensor_scalar is_equal (4x mode)
                                       #   or GpSimd local_scatter (2 halves)
    aggT  = sum_t stage_t.T @ O_t      # PE, fp8 lhsT x bf16 rhs, PSUM f32
    updT  = W @ aggT                   # PE (replicated W, bf16)
    updT += b                          # ACT Identity+bias -> bf16
    outT[:, w] = updT                  # DMA out per window group
Nodes with indeg == 0 (expected ~0 of 40000 at mean indeg 16) keep h;
the host patches them after the gather, along with any window-capacity
spill (6-sigma event) recomputed exactly on the host.
"""

import sys

sys.path.insert(0, "/opt/trn_rl_repo")

import numpy as np
import ml_dtypes

import concourse.bacc as bacc
import concourse.mybir as mybir
import concourse.tile as tile
from concourse.bass_utils import run_bass_kernel_spmd

BF16 = ml_dtypes.bfloat16
FP8 = ml_dtypes.float8_e4m3

# problem geometry (hardcoded per spec)
N_NODES = 40000
N_EDGES = 640000
HID = 128
P = 128

N_CORES = 8
PAD_NODES = 40960           # 8 cores x 40 windows x 128 nodes
NPC = PAD_NODES // N_CORES  # 5120 nodes per core
WPC = NPC // P              # 40 windows per core
T = 17                      # edge tiles per window (capacity T*128 = 2176, mean 2048)
THA = 9                     # tiles in one-hot half A (gpsimd path)
THB = T - THA
NIXA = THA + 1              # local_scatter num_idxs, half A (even)
NIXB = THB
NIX2 = NIXA + NIXB
GRP = 2                     # windows fused per Linear/bias batch (256 cols)

_NC_CACHE = {}


def _dve_window(w: int) -> bool:
    """One-hot engine split: 27 windows on DVE (2x mode), 13 on GpSimd."""
    return w % 3 != 1


def _build_nc():
    key = "v8"
    if key in _NC_CACHE:
        return _NC_CACHE[key]
    f32 = mybir.dt.float32
    bf16 = mybir.dt.bfloat16
    fp8 = mybir.dt.float8e4
    i16 = mybir.dt.int16
    nc = bacc.Bacc(None, target_bir_lowering=False)

    stage = nc.declare_dram_parameter("stage", [P, WPC * T * P], fp8, isOutput=False)
    colix = nc.declare_dram_parameter("colix", [P, WPC * NIX2], i16, isOutput=False)
    # dst-local per (partition, window, tile), duplicated x2 so the one-hot
    # compare's in0 has an innermost step-1 pair -> DVE 2x_1p mode
    dl = nc.declare_dram_parameter("dl", [P, WPC * T * 2], bf16, isOutput=False)
    iota = nc.declare_dram_parameter("iota", [P, P], bf16, isOutput=False)
    wt = nc.declare_dram_parameter("wt", [P, P], bf16, isOutput=False)
    b2 = nc.declare_dram_parameter("b2", [P, 1], f32, isOutput=False)
    outT = nc.declare_dram_parameter("outT", [P, NPC], bf16, isOutput=True)

    with tile.TileContext(nc) as tc:
        with (
            tc.tile_pool(name="const", bufs=1) as constp,
            tc.tile_pool(name="stagep", bufs=6) as stagep,
            tc.tile_pool(name="onehotp", bufs=4) as onehotp,
            tc.tile_pool(name="smallp", bufs=6) as smallp,
            tc.tile_pool(name="psA", bufs=4, space="PSUM") as psA,
            tc.tile_pool(name="psB", bufs=2, space="PSUM") as psB,
        ):
            wt_t = constp.tile([P, P], bf16)
            nc.sync.dma_start(out=wt_t[:], in_=wt[:])
            b2_t = constp.tile([P, 1], f32)
            nc.sync.dma_start(out=b2_t[:], in_=b2[:])
            ones_t = constp.tile([P, NIXA], bf16)
            nc.vector.memset(ones_t[:], 1.0)
            cix_t = constp.tile([P, WPC * NIX2], i16)
            nc.sync.dma_start(out=cix_t[:], in_=colix[:])
            iota_t = constp.tile([P, P], bf16)
            nc.sync.dma_start(out=iota_t[:], in_=iota[:])
            dl_t = constp.tile([P, WPC * T * 2], bf16)
            nc.sync.dma_start(out=dl_t[:], in_=dl[:])

            for w in range(WPC):
                if w % 2 == 0:
                    st2 = stagep.tile([P, 2 * T * P], fp8, tag="stage")
                    nc.sync.dma_start(
                        out=st2[:], in_=stage[:, w * T * P : (w + 2) * T * P]
                    )
                st = st2[:, (w % 2) * T * P : (w % 2 + 1) * T * P]

                oh = onehotp.tile([P, T * P], bf16, tag="oh")
                if _dve_window(w):
                    # oh[p, t, j, i] = (dl[p, w, t] == 2*j + i): all operands
                    # have innermost [1, 2] APs -> 2x_1p DVE mode
                    nc.vector.tensor_tensor(
                        out=oh[:].rearrange("p (t j i) -> p t j i", j=P // 2, i=2),
                        in0=dl_t[
                            :, w * T * 2 : (w + 1) * T * 2
                        ].rearrange("p (t i) -> p t i", i=2)[:, :, None, :]
                        .to_broadcast([P, T, P // 2, 2]),
                        in1=iota_t[:].rearrange("p (j i) -> p j i", i=2)[:, None, :, :]
                        .to_broadcast([P, T, P // 2, 2]),
                        op=mybir.AluOpType.is_equal,
                    )
                else:
                    for thn, base_ix, nix, off in (
                        (THA, 0, NIXA, 0),
                        (THB, NIXA, NIXB, THA * P),
                    ):
                        nc.gpsimd.local_scatter(
                            out_ap=oh[:, off : off + thn * P],
                            data_ap=ones_t[:, :nix],
                            idxs_ap=cix_t[
                                :, w * NIX2 + base_ix : w * NIX2 + base_ix + nix
                            ],
                            channels=P,
                            num_elems=thn * P,
                            num_idxs=nix,
                        )

                paggT = psA.tile([P, P], f32, tag="paggT")
                for t in range(T):
                    nc.tensor.matmul(
                        out=paggT[:],
                        lhsT=st[:, t * P : (t + 1) * P],
                        rhs=oh[:, t * P : (t + 1) * P],
                        start=(t == 0),
                        stop=(t == T - 1),
                    )
                wi = w % GRP
                if wi == 0:
                    aggT4 = smallp.tile([P, GRP * P], bf16, tag="aggT")
                nc.scalar.copy(out=aggT4[:, wi * P : (wi + 1) * P], in_=paggT[:])

                if wi == GRP - 1:
                    g0 = (w - GRP + 1) * P
                    pupdT = psB.tile([P, GRP * P], f32, tag="pupdT")
                    nc.tensor.matmul(
                        out=pupdT[:], lhsT=wt_t[:], rhs=aggT4[:], start=True, stop=True
                    )
                    updT_s = smallp.tile([P, GRP * P], bf16, tag="updT")
                    nc.scalar.activation(
                        out=updT_s[:],
                        in_=pupdT[:],
                        func=mybir.ActivationFunctionType.Identity,
                        bias=b2_t[:, :1],
                    )
                    nc.sync.dma_start(
                        out=outT[:, g0 : g0 + GRP * P], in_=updT_s[:]
                    )

    nc.finalize()
    _NC_CACHE[key] = nc
    return nc


def kernel(h, W, b, src, dst):
    h = np.ascontiguousarray(np.asarray(h, dtype=np.float32))
    W = np.ascontiguousarray(np.asarray(W, dtype=np.float32))
    b = np.ascontiguousarray(np.asarray(b, dtype=np.float32))
    src = np.asarray(src).astype(np.int64)
    dst = np.asarray(dst).astype(np.int64)
    n, hid = h.shape
    assert (n, hid) == (N_NODES, HID)

    # ---- host-side sharding: bucket edges by dst window, fixed-capacity slots
    order = np.argsort(dst, kind="stable")
    dst_s = dst[order]
    src_s = src[order]
    win_bounds = np.searchsorted(dst_s, np.arange(0, PAD_NODES + P, P))
    cap = T * P
    n_win = PAD_NODES // P  # 320

    # fp8 quantization of gathered rows with per-destination error diffusion:
    # within each dst's contiguous run of edges, quantize v + carry and push
    # the residual onto the next edge; the run's sum error telescopes to the
    # final carry (~one fp8 step) instead of accumulating across edges.
    vals = h[src_s]  # [E, HID] f32, dst-sorted
    starts = np.searchsorted(dst_s, np.arange(N_NODES))
    counts = np.bincount(dst_s, minlength=N_NODES)
    q = np.empty((N_EDGES, HID), FP8)
    carry = np.zeros((N_NODES, HID), np.float32)
    for k in range(int(counts.max())):
        sel = counts > k
        pos = starts[sel] + k
        v = vals[pos] + carry[sel]
        qk = v.astype(FP8)
        q[pos] = qk
        carry[sel] = v - qk.astype(np.float32)

    spill_nodes = []
    slot_q = np.zeros((n_win, cap, HID), FP8)
    slot_dl = np.full((n_win, cap), -1, np.int64)
    for wgl in range(n_win):
        lo, hi = win_bounds[wgl], win_bounds[wgl + 1]
        cnt = hi - lo
        take = min(cnt, cap)
        slot_q[wgl, :take] = q[lo : lo + take]
        slot_dl[wgl, :take] = dst_s[lo : lo + take] - wgl * P
        if cnt > cap:
            spill_nodes.append(np.unique(dst_s[lo + cap : hi]))

    indeg = np.bincount(dst, minlength=PAD_NODES)

    # one-hot column indices per slot (gpsimd local_scatter path):
    # col = (tile % TH) * 128 + dst_local, per (window, half)
    sl = slot_dl.reshape(n_win, T, P)
    colix_all = np.full((n_win, NIX2, P), -1, np.int64)
    tlA = sl[:, :THA, :]
    colix_all[:, :THA, :] = np.where(
        tlA >= 0, (np.arange(THA)[None, :, None]) * P + tlA, -1
    )
    tlB = sl[:, THA:, :]
    colix_all[:, NIXA : NIXA + THB, :] = np.where(
        tlB >= 0, (np.arange(THB)[None, :, None]) * P + tlB, -1
    )

    WT = np.ascontiguousarray(W.T).astype(BF16)
    b2 = np.ascontiguousarray(b[:, None])
    iota_np = np.tile(np.arange(P, dtype=np.float32), (P, 1)).astype(BF16)

    in_maps = []
    for c in range(N_CORES):
        wsl = slice(c * WPC, (c + 1) * WPC)
        rows = slot_q[wsl].reshape(WPC, T, P, HID)
        stage_np = np.ascontiguousarray(
            rows.transpose(2, 0, 1, 3).reshape(P, WPC * T * P)
        )
        colix_np = np.ascontiguousarray(
            colix_all[wsl].transpose(2, 0, 1).reshape(P, WPC * NIX2)
        ).astype(np.int16)
        dl_win = (
            np.where(slot_dl[wsl] >= 0, slot_dl[wsl], 255)
            .reshape(WPC, T, P)
            .transpose(2, 0, 1)
            .astype(np.float32)
        )  # [P, WPC, T]
        dl_np = np.ascontiguousarray(
            np.repeat(dl_win.reshape(P, WPC * T), 2, axis=1)
        ).astype(BF16)
        in_maps.append(
            {
                "stage": stage_np,
                "colix": colix_np,
                "dl": dl_np,
                "iota": iota_np,
                "wt": WT,
                "b2": b2,
            }
        )

    nc = _build_nc()
    res = run_bass_kernel_spmd(nc, in_maps, core_ids=list(range(N_CORES)))

    out = np.concatenate(
        [res.results[c]["outT"].T.astype(np.float32) for c in range(N_CORES)], axis=0
    )
    out = np.ascontiguousarray(out[:N_NODES])

    # nodes with no incoming edge keep their input feature
    zi = np.flatnonzero(indeg[:N_NODES] == 0)
    if zi.size:
        out[zi] = h[zi]

    # ---- host patch for (statistically negligible) window-capacity spill
    if spill_nodes:
        nodes = np.unique(np.concatenate(spill_nodes))
        nodes = nodes[nodes < N_NODES]
        if nodes.size:
            sel = np.isin(dst, nodes)
            agg = np.zeros((nodes.size, HID), np.float32)
            remap = {int(v): i for i, v in enumerate(nodes)}
            np.add.at(agg, [remap[int(d)] for d in dst[sel]], h[src[sel]])
            out[nodes] = agg @ W.T + b

    return out
